# revision 19
# baseline (speedup 1.0000x reference)
"""Sliding-window causal attention (window=1024) for B=2,T=2048,H=16,D=128 fp32
on 8 trn2 NeuronCores. Shards the 32 (batch, head) pairs 4-per-core.

v2: per (b,h): S^T = K @ Q^T blockwise with block-stationary wide matmuls into
1536-col PSUM units (one exp ACTIVATE per unit), es in fp8e4 (biased to fit
range) so PV stationary loads hit 4x FWL, sliding-window masks as post-exp
DVE multiplies, Q/K transposed via the DMA xbar instead of the PE, PV as
per-tile 9-matmul accumulation chains with [V | ones] moving operand so the
softmax denominator lands in column 128 of the same PSUM bank.
"""
import math

import numpy as np

import concourse.bacc as bacc
import concourse.mybir as mybir
from concourse import tile
from concourse.bass_utils import run_bass_kernel_spmd

B, T, H, D = 2, 2048, 16, 128
WINDOW = 1024
NCORES = 8
BH = B * H                  # 32 (b,h) pairs
BH_PER_CORE = BH // NCORES  # 4
NT = T // 128               # 16 seq tiles
G = 4                       # q-tiles per group (512 queries)
NG = NT // G
WB = WINDOW // 128          # window in blocks
UNIT = 12                   # tiles per st/exp unit (1536 cols = 3 PSUM banks)

f32 = mybir.dt.float32
bf16 = mybir.dt.bfloat16
fp8 = mybir.dt.float8e4
AF = mybir.ActivationFunctionType
ALU = mybir.AluOpType

ES_DTYPE = bf16      # fp8e4 halves PV LDWEIGHTS but costs 2x the err budget
V_DTYPE = bf16       # moving operand dtype for PV
EXP_BIAS = 0.0


def group_plan(g):
    """Block-major (b, t) tile list for group g, packed into units of UNIT
    tiles. Returns (units, pos) where units = list of list of (b, t), and
    pos[(b, t)] = (unit_idx, col_offset)."""
    tiles = []
    for b in range(max(0, G * g - WB), G * g + G):
        for t in range(max(G * g, b), min(G * g + G, b + WB + 1)):
            tiles.append((b, t))
    units, pos = [], {}
    for i, (b, t) in enumerate(tiles):
        u, j = divmod(i, UNIT)
        if j == 0:
            units.append([])
        units[u].append((b, t))
        pos[(b, t)] = (u, j * 128)
    return units, pos


def unit_segments(unit):
    """Greedy segments within a unit: consecutive tiles sharing the same
    block b, split at 512-col PSUM bank boundaries. Returns list of
    (b, t0, t1, col_off)."""
    segs = []
    for j, (b, t) in enumerate(unit):
        if (segs and segs[-1][0] == b and segs[-1][2] == t - 1
                and (j % 4) != 0):
            bb, t0, _, off = segs.pop()
            segs.append((bb, t0, t, off))
        else:
            segs.append((b, t, t, j * 128))
    return segs


def build_nc(n_bh=BH_PER_CORE):
    nc = bacc.Bacc()
    q = nc.declare_dram_parameter("q", [n_bh, T, D], f32, isOutput=False)
    k = nc.declare_dram_parameter("k", [n_bh, T, D], f32, isOutput=False)
    v = nc.declare_dram_parameter("v", [n_bh, T, D], f32, isOutput=False)
    o = nc.declare_dram_parameter("o", [n_bh, T, D], f32, isOutput=True)

    scale = 1.0 / math.sqrt(D)

    with tile.TileContext(nc) as tc:
        with (
            tc.tile_pool(name="const", bufs=1) as constp,
            tc.tile_pool(name="io", bufs=2) as iop,
            tc.tile_pool(name="qt", bufs=2) as qtp,
            tc.tile_pool(name="es", bufs=6) as esp,
            tc.tile_pool(name="outp", bufs=3) as outp,
            tc.tile_pool(name="ps_st", bufs=2, space="PSUM") as ps_st,
            tc.tile_pool(name="ps_o", bufs=2, space="PSUM") as ps_o,
        ):
            # --- loads: fp32->dtype casts on SWDGE; v gets a ones column
            # at col 128 so PV also accumulates the softmax denominator.
            def issue_loads(bh, chunked=False):
                qb = iop.tile([128, NT, 128], bf16, tag="qb", name=f"qb_{bh}")
                kb = iop.tile([128, NT, 128], bf16, tag="kb", name=f"kb_{bh}")
                v8 = iop.tile([128, NT, 130], V_DTYPE, tag="v8",
                              name=f"v8_{bh}")
                if chunked:
                    for src_p, dst in ((q, qb), (k, kb)):
                        full = src_p[bh].rearrange("(n p) d -> p n d", p=128)
                        nc.gpsimd.dma_start(out=dst[:, 0:4, :],
                                            in_=full[:, 0:4, :])
                    return qb, kb, v8
                nc.gpsimd.dma_start(
                    out=qb[:],
                    in_=q[bh].rearrange("(n p) d -> p n d", p=128))
                nc.gpsimd.dma_start(
                    out=kb[:],
                    in_=k[bh].rearrange("(n p) d -> p n d", p=128))
                nc.gpsimd.dma_start(
                    out=v8[:, :, 0:128],
                    in_=v[bh].rearrange("(n p) d -> p n d", p=128))
                nc.gpsimd.memset(v8[:, :, 128:129], 1.0)
                return qb, kb, v8

            def issue_loads_rest(bh, qb, kb, v8):
                for src_p, dst in ((q, qb), (k, kb)):
                    full = src_p[bh].rearrange("(n p) d -> p n d", p=128)
                    nc.gpsimd.dma_start(out=dst[:, 4:NT, :],
                                        in_=full[:, 4:NT, :])
                nc.gpsimd.dma_start(
                    out=v8[:, :, 0:128],
                    in_=v[bh].rearrange("(n p) d -> p n d", p=128))
                nc.gpsimd.memset(v8[:, :, 128:129], 1.0)

            def alloc_qtkt(bh):
                qt = qtp.tile([128, NT, 128], bf16, tag="qt", name=f"qt_{bh}")
                kt = qtp.tile([128, NT, 128], bf16, tag="kt", name=f"kt_{bh}")
                return qt, kt

            def transpose_quad(bh, qb, kb, qt, kt, quad):
                """PE-transpose 4 tiles of qb and kb into qt/kt, borrowing a
                PSUM bank from the ps_o pool."""
                for src_t, dst in ((qb, qt), (kb, kt)):
                    trt = ps_o.tile([128, 512], f32, tag="ot",
                                    name=f"tr_{bh}_{quad}")
                    tr = trt.bitcast(bf16)
                    for i in range(4):
                        n = quad * 4 + i
                        nc.tensor.matmul(
                            tr[:, i * 128:(i + 1) * 128],
                            src_t[:, n, :], ident[:],
                            is_transpose=True,
                            start=(i == 0), stop=(i == 3),
                            skip_group_check=True)
                    nc.vector.tensor_copy(
                        dst[:, quad * 4:quad * 4 + 4, :], tr[:, 0:512])

            plans = [group_plan(g) for g in range(NG)]

            # first load chunks go to the SWDGE queue before anything else
            loaded = {0: issue_loads(0, chunked=True)}

            # --- constants: diag / anti-diag keep-masks (in es dtype)
            ones_f = constp.tile([128, 128], f32)
            mdiag_f = constp.tile([128, 128], f32)
            madiag_f = constp.tile([128, 128], f32)
            nc.gpsimd.memset(ones_f[:], 1.0)
            # diag keep-mask (allowed k <= q): keep where col - p >= 0
            nc.gpsimd.affine_select(
                out=mdiag_f[:], in_=ones_f[:], compare_op=ALU.is_ge,
                fill=0.0, base=0, channel_multiplier=-1, pattern=[[1, 128]],
            )
            # anti-diag keep-mask (allowed k > q): keep where p - col - 1 >= 0
            nc.gpsimd.affine_select(
                out=madiag_f[:], in_=ones_f[:], compare_op=ALU.is_ge,
                fill=0.0, base=-1, channel_multiplier=1, pattern=[[-1, 128]],
            )
            ident_f = constp.tile([128, 128], f32)
            nc.gpsimd.affine_select(
                out=ident_f[:], in_=ones_f[:], compare_op=ALU.is_equal,
                fill=0.0, base=0, channel_multiplier=1, pattern=[[-1, 128]],
            )
            mdiag = constp.tile([128, 128], ES_DTYPE)
            madiag = constp.tile([128, 128], ES_DTYPE)
            ident = constp.tile([128, 128], bf16)
            nc.vector.tensor_copy(mdiag[:], mdiag_f[:])
            nc.vector.tensor_copy(madiag[:], madiag_f[:])
            nc.vector.tensor_copy(ident[:], ident_f[:])

            # rest of bh0's inputs behind the first chunks + consts
            issue_loads_rest(0, *loaded[0])

            def emit_qk(qt, kt, unit, es_units):
                ncols = len(unit) * 128
                st = ps_st.tile([128, 1536], f32, tag="st")
                es = esp.tile([128, 1536], ES_DTYPE, tag="es")
                for (b, t0, t1, off) in unit_segments(unit):
                    w = (t1 - t0 + 1) * 128
                    nc.tensor.matmul(
                        st[:, off:off + w], kt[:, b, :],
                        qt[:, t0:t1 + 1, :],
                        start=True, stop=True,
                        skip_group_check=True)
                nc.scalar.activation(
                    es[:, 0:ncols], st[:, 0:ncols],
                    AF.Exp, scale=scale)
                # post-exp sliding-window masks on the DVE
                for j, (b, t) in enumerate(unit):
                    c = j * 128
                    if b == t:
                        nc.vector.tensor_mul(
                            es[:, c:c + 128], es[:, c:c + 128], mdiag[:])
                    elif b == t - WB:
                        nc.vector.tensor_mul(
                            es[:, c:c + 128], es[:, c:c + 128], madiag[:])
                es_units.append(es)

            def emit_pv(bh, v8, tiles, es_units, pos, g, oo, rcp, nleft):
                # interleave pairs of per-tile chains across the two O banks
                # so consecutive matmuls never accumulate to the same region
                for i0 in range(0, len(tiles), 2):
                    pair = tiles[i0:i0 + 2]
                    chains = [(t, list(range(max(0, t - WB), t + 1)),
                               ps_o.tile([128, 512], f32, tag="ot",
                                         name=f"ot_{bh}_{g}_{t}"))
                              for t in pair]
                    depth = max(len(c[1]) for c in chains)
                    for j in range(depth):
                        for (t, blocks, ot) in chains:
                            if j < len(blocks):
                                ub, cb = pos[(blocks[j], t)]
                                nc.tensor.matmul(
                                    ot[:, 0:129],
                                    es_units[ub][:, cb:cb + 128],
                                    v8[:, blocks[j], 0:129],
                                    start=(j == 0),
                                    stop=(j == len(blocks) - 1))
                    for (t, blocks, ot) in chains:
                        i = t - G * g
                        nc.vector.reciprocal(rcp[:, i:i + 1], ot[:, 128:129])
                        nc.vector.tensor_scalar_mul(
                            oo[:, i, :], ot[:, 0:128], rcp[:, i:i + 1])
                        nleft[0] -= 1
                        if nleft[0] == 0:
                            nc.sync.dma_start(
                                out=o[bh, 512 * g:512 * (g + 1), :].rearrange(
                                    "(t p) d -> p t d", p=128),
                                in_=oo[:])

            # deferred PV work, pipelined across groups AND bh boundaries
            pending = []
            nxt = None
            for bh in range(n_bh):
                qb, kb, v8 = loaded.pop(bh)
                if bh == 0:
                    qt, kt = alloc_qtkt(0)
                    for quad in range(2):
                        transpose_quad(0, qb, kb, qt, kt, quad)
                else:
                    qt, kt = nxt
                if bh + 1 < n_bh:
                    loaded[bh + 1] = issue_loads(bh + 1)
                    nxt = alloc_qtkt(bh + 1)

                for g in range(NG):
                    units, pos = plans[g]
                    es_units = []
                    rcp = outp.tile([128, G], f32, tag="rcp")
                    oo = outp.tile([128, G, 128], f32, tag="oo")
                    nleft = [G]
                    for u, unit in enumerate(units):
                        emit_qk(qt, kt, unit, es_units)
                        compl = [t for (b, t) in unit if b == t]
                        pending.append((bh, v8, compl, es_units, pos, g, oo,
                                        rcp, nleft))
                        if len(pending) >= 2:
                            emit_pv(*pending.pop(0))
                    if bh == 0 and g <= 1:
                        transpose_quad(0, qb, kb, qt, kt, g + 2)
                    if g >= 1 and bh + 1 < n_bh:
                        nqb, nkb, _ = loaded[bh + 1]
                        transpose_quad(bh + 1, nqb, nkb, nxt[0], nxt[1],
                                       g - 1)
                if bh + 1 < n_bh:
                    nqb, nkb, _ = loaded[bh + 1]
                    transpose_quad(bh + 1, nqb, nkb, nxt[0], nxt[1], 3)
            while pending:
                emit_pv(*pending.pop(0))

    if not nc.is_finalized():
        nc.finalize()
    return nc


_nc = None


def _get_nc():
    global _nc
    if _nc is None:
        _nc = build_nc()
    return _nc


def make_in_maps(q, k, v):
    q = np.ascontiguousarray(np.asarray(q, dtype=np.float32))
    k = np.ascontiguousarray(np.asarray(k, dtype=np.float32))
    v = np.ascontiguousarray(np.asarray(v, dtype=np.float32))
    # [B, T, H, D] -> [B*H, T, D]
    qs = np.ascontiguousarray(q.transpose(0, 2, 1, 3).reshape(BH, T, D))
    ks = np.ascontiguousarray(k.transpose(0, 2, 1, 3).reshape(BH, T, D))
    vs = np.ascontiguousarray(v.transpose(0, 2, 1, 3).reshape(BH, T, D))
    return [
        {
            "q": qs[c * BH_PER_CORE:(c + 1) * BH_PER_CORE],
            "k": ks[c * BH_PER_CORE:(c + 1) * BH_PER_CORE],
            "v": vs[c * BH_PER_CORE:(c + 1) * BH_PER_CORE],
        }
        for c in range(NCORES)
    ]


def assemble_out(results):
    out = np.empty((BH, T, D), np.float32)
    for c in range(NCORES):
        out[c * BH_PER_CORE:(c + 1) * BH_PER_CORE] = results[c]["o"]
    return np.ascontiguousarray(
        out.reshape(B, H, T, D).transpose(0, 2, 1, 3))


def kernel(q, k, v, window_size):
    assert int(window_size) == WINDOW
    in_maps = make_in_maps(q, k, v)
    res = run_bass_kernel_spmd(_get_nc(), in_maps, list(range(NCORES))).results
    return assemble_out(res)
